# revision 15
# baseline (speedup 1.0000x reference)
"""Trainium2 Bass kernel: collaborative-filtering score (segment_reduce).

Math (per batch element b):
    ubf[u]   = masked mean over nonzero entries of rating_mtx[u, :]
    score[b] = sum_u S[user_b, u] * (R[u, item_b] - ubf[u])
    out[b]   = 5 * sigmoid(score[b] + user_bias[user_b] + item_bias[item_b] + gb)

Distribution: BATCH-sharded, TWO launches.

Launch 1 (ubf): user axis sharded 8-way; each core scans its private
[4096 items x 1024 users] fp8 slice of (R.T - 2.5) and produces
-(ubf - 2.5) for its users via PE masked sum/count in fp32 PSUM. The 8
x [1024] slices are concatenated and re-laid-out on the HOST (pure
unshard/reshard of 32KB). A single-launch on-device exchange was built
and measured first: the collectives stack costs ~110us for the
auto-inserted kernel-entry barrier plus ~70us per op on this 8-core
mesh, so any collective-dependent compute cannot start before ~200us --
slower than this entire kernel.

Launch 2 (main): batch sharded 8-way. Core k owns batch slice
[k*1024,(k+1)*1024) and transpose-gathers FULL 8192-user rows per
(user_b, item_b) pair: G (bf16, three contiguous tables: the gather
ucode requires rows <=16KB and contiguous elem_step) and A (fp8 pairs
packed in int16, exact values +-{0.5,1.5,2.5}). Per 256-idx chunk, one
fused fp32-PSUM reduction accumulates [ubf-weighted G terms + hi/lo
bias columns + ones-weighted A(x)G products]; the product pass runs on
DVE (mixed fp8*bf16, single bf16 rounding) in place over gk after the
ubf pass has consumed it. sigmoid*5, DMA out the [1024] slice; the host
concatenates the 8 slices. Only 2048+few gather descriptors per core
(vs 16384 user-sharded), so GPSIMD descriptor generation is off the
critical path and the gathers stream at ~300GB/s.
"""

import sys
from dataclasses import dataclass

import numpy as np

if "/opt/trn_rl_repo" not in sys.path:
    sys.path.insert(0, "/opt/trn_rl_repo")


@dataclass(frozen=True)
class Cfg:
    n_users: int = 8192
    n_items: int = 4096
    batch: int = 8192
    n_cores: int = 8
    ch: int = 256  # gather idxs per chunk

    @property
    def bl(self) -> int:  # batch per core
        return self.batch // self.n_cores

    @property
    def ul(self) -> int:  # users per core (ubf launch)
        return self.n_users // self.n_cores

    @property
    def gfg(self) -> int:  # G data f-groups
        return self.n_users // 128

    @property
    def afu(self) -> int:  # A data unit f-groups (i16 units)
        return self.n_users // 256

    @property
    def awu(self) -> int:  # A row width (i16 units): data + 128-unit bias blk
        return self.n_users // 2 + 128


def build_ubf_program(cfg: Cfg):
    from concourse import bacc, mybir, tile

    f32 = mybir.dt.float32
    i16 = mybir.dt.int16
    fp8 = mybir.dt.float8e4
    Alu = mybir.AluOpType

    I, UL = cfg.n_items, cfg.ul
    ICH = I // 128
    USL = UL // 2  # slice width in i16 units

    nc = bacc.Bacc(None, target_bir_lowering=False, debug=False)
    rsl_t = nc.dram_tensor("ratt_slice", [I, USL], i16, kind="ExternalInput")
    out_t = nc.dram_tensor("ubf", [1, UL], f32, kind="ExternalOutput")

    with tile.TileContext(nc) as tc:
        with (
            tc.tile_pool(name="static", bufs=1) as st,
            tc.tile_pool(name="rstream", bufs=4) as rpool,
            tc.tile_pool(name="maskp", bufs=2) as mpool,
            tc.tile_pool(name="ps", bufs=1, space="PSUM") as pp,
        ):
            ones8 = st.tile([128, 1], fp8)
            nc.gpsimd.memset(ones8[:], 1.0)
            ps_s = [pp.tile([1, 512], f32, name=f"ps_s{j}") for j in range(2)]
            ps_c = [pp.tile([1, 512], f32, name=f"ps_c{j}") for j in range(2)]
            for g in range(ICH):
                rt = rpool.tile([128, USL], i16, name="rt")
                nc.sync.dma_start(out=rt[:], in_=rsl_t[g * 128 : (g + 1) * 128, :])
                atf = rt[:].bitcast(fp8)  # [128, 1024] users in natural order
                mk = mpool.tile([128, UL], fp8, name="mk")
                nc.vector.tensor_scalar(
                    out=mk[:], in0=atf, scalar1=-2.5, scalar2=None,
                    op0=Alu.not_equal,
                )
                first, last = g == 0, g == ICH - 1
                for j in range(2):
                    nc.tensor.matmul(
                        out=ps_s[j][:], lhsT=ones8[:],
                        rhs=atf[:, j * 512 : (j + 1) * 512],
                        start=first, stop=last,
                    )
                    nc.tensor.matmul(
                        out=ps_c[j][:], lhsT=ones8[:],
                        rhs=mk[:, j * 512 : (j + 1) * 512],
                        start=first, stop=last,
                    )

            # table is pre-centered: sum_A = sum(R) - 2.5*I, so
            # -(ubf - 2.5) = -((sum_A + 2.5*I)/max(cnt,1) - 2.5); cnt==0
            # rows give +2.5 = -(0 - 2.5), matching the reference's ubf=0.
            ubf_loc = st.tile([1, UL], f32)
            rcp = st.tile([1, UL], f32)
            for j in range(2):
                sl = ubf_loc[:, j * 512 : (j + 1) * 512]
                rc = rcp[:, j * 512 : (j + 1) * 512]
                nc.vector.tensor_scalar(
                    out=rc, in0=ps_c[j][:], scalar1=1.0, scalar2=None,
                    op0=Alu.max,
                )
                nc.vector.reciprocal(out=rc, in_=rc)
                nc.vector.scalar_tensor_tensor(
                    out=rc, in0=ps_s[j][:], scalar=2.5 * I, in1=rc,
                    op0=Alu.add, op1=Alu.mult,
                )
                nc.vector.tensor_scalar(
                    out=sl, in0=rc, scalar1=2.5, scalar2=-1.0,
                    op0=Alu.subtract, op1=Alu.mult,
                )
            nc.sync.dma_start(out=out_t[:], in_=ubf_loc[:])

    nc.compile()
    return nc


def build_main_program(cfg: Cfg):
    from concourse import bacc, mybir, tile

    f32 = mybir.dt.float32
    i16 = mybir.dt.int16
    bf16 = mybir.dt.bfloat16
    fp8 = mybir.dt.float8e4
    Alu = mybir.AluOpType
    Act = mybir.ActivationFunctionType

    U, I, BL, CH = cfg.n_users, cfg.n_items, cfg.bl, cfg.ch
    GFG, AFU, AWU = cfg.gfg, cfg.afu, cfg.awu
    NCH = BL // CH
    IDXC = BL // 16

    nc = bacc.Bacc(
        None, target_bir_lowering=False, debug=False, num_swdge_queues=2
    )

    glo_t = nc.dram_tensor("g_lo", [U, U // 2], bf16, kind="ExternalInput")
    ghi_t = nc.dram_tensor("g_hi", [U, U // 2], bf16, kind="ExternalInput")
    gbi_t = nc.dram_tensor("g_bias", [U, 128], bf16, kind="ExternalInput")
    rtt_t = nc.dram_tensor("ratt_aug", [I, AWU], i16, kind="ExternalInput")
    ubfc_t = nc.dram_tensor("ubf_ct", [128, GFG], bf16, kind="ExternalInput")
    uidx_t = nc.dram_tensor("uidx", [128, IDXC], i16, kind="ExternalInput")
    iidx_t = nc.dram_tensor("iidx", [128, IDXC], i16, kind="ExternalInput")
    out_t = nc.dram_tensor("out", [BL], f32, kind="ExternalOutput")

    with tile.TileContext(nc) as tc:
        with (
            tc.tile_pool(name="static", bufs=1) as st,
            tc.tile_pool(name="gpool", bufs=NCH) as gpool,
            tc.tile_pool(name="apool", bufs=2) as apool,
            tc.tile_pool(name="psum", bufs=2, space="PSUM") as pp,
        ):
            # ---- statics ----
            ones_b = st.tile([128, 1], bf16)
            nc.gpsimd.memset(ones_b[:], 1.0)
            uidx_sb = st.tile([128, IDXC], i16)
            nc.sync.dma_start(out=uidx_sb[:], in_=uidx_t[:])
            iidx_sb = st.tile([128, IDXC], i16)
            nc.sync.dma_start(out=iidx_sb[:], in_=iidx_t[:])
            ubf_ct = st.tile([128, GFG], bf16)
            nc.sync.dma_start(out=ubf_ct[:], in_=ubfc_t[:])

            # ---- gathers: all G first (gpool holds every chunk, queue 0
            # never blocks), then A (ak(k>=2) reuses a buffer and its
            # descriptor generation waits on prod(k-2)) ----
            gks, aks = [], []
            icn = CH // 16
            H = GFG // 2
            for k in range(NCH):
                gk = gpool.tile([128, GFG + 1, CH], bf16, name="gk")
                idx = uidx_sb[:, k * icn : (k + 1) * icn]
                nc.gpsimd.dma_gather(
                    out_ap=gk[:, 0:H, :], in_ap=glo_t[:], idxs_ap=idx,
                    num_idxs=CH, num_idxs_reg=CH, elem_size=U // 2,
                    transpose=True, queue_num=0,
                )
                nc.gpsimd.dma_gather(
                    out_ap=gk[:, H : 2 * H, :], in_ap=ghi_t[:], idxs_ap=idx,
                    num_idxs=CH, num_idxs_reg=CH, elem_size=U // 2,
                    transpose=True, queue_num=0,
                )
                nc.gpsimd.dma_gather(
                    out_ap=gk[:, GFG : GFG + 1, :], in_ap=gbi_t[:], idxs_ap=idx,
                    num_idxs=CH, num_idxs_reg=CH, elem_size=128,
                    transpose=True, queue_num=0,
                )
                gks.append(gk)
            for k in range(NCH):
                ak = apool.tile([128, AFU + 1, CH], i16, name="ak")
                nc.gpsimd.dma_gather(
                    out_ap=ak[:], in_ap=rtt_t[:],
                    idxs_ap=iidx_sb[:, k * icn : (k + 1) * icn],
                    num_idxs=CH, num_idxs_reg=CH, elem_size=AWU,
                    transpose=True, queue_num=1,
                )
                aks.append(ak)

            # ---- main loop ----
            # per chunk: ps2[1,CH] += sum_u -ubf_c[u]*G + bias columns (PE,
            # reads raw gk); then p = A(x)G overwrites gk in place (DVE);
            # then ps1[1,2*CH] += fg-pair-merged product sums (PE). Emission
            # is software-pipelined so P2(k+1) runs while DVE computes p(k).
            scores_row = st.tile([1, BL], f32)
            ps1_k, ps2_k = [None] * NCH, [None] * NCH

            def emit_pass2(k):
                gk, ak = gks[k], aks[k]
                ps2 = pp.tile([1, CH], f32, name="ps2")
                ps2_k[k] = ps2
                for fg in range(GFG):
                    nc.tensor.matmul(
                        out=ps2[:], lhsT=ubf_ct[:, fg : fg + 1],
                        rhs=gk[:, fg, :], start=(fg == 0), stop=False,
                    )
                nc.tensor.matmul(
                    out=ps2[:], lhsT=ones_b[:], rhs=gk[:, GFG, :],
                    start=False, stop=False,
                )
                nc.tensor.matmul(
                    out=ps2[:], lhsT=ones_b[:],
                    rhs=ak[:, AFU, :].bitcast(bf16),
                    start=False, stop=True,
                )

            def emit_prod(k):
                gk, ak = gks[k], aks[k]
                akf = ak[:, 0:AFU, :].bitcast(fp8).rearrange(
                    "p f (i b) -> p f b i", b=2
                )
                gkv = gk[:, 0:GFG, :].rearrange("p (f b) i -> p f b i", b=2)
                nc.vector.tensor_tensor(out=gkv, in0=akf, in1=gkv, op=Alu.mult)

            def emit_pass1(k):
                gk = gks[k]
                ps1 = pp.tile([1, 2 * CH], f32, name="ps1")
                ps1_k[k] = ps1
                for f2 in range(GFG // 2):
                    nc.tensor.matmul(
                        out=ps1[:],
                        lhsT=ones_b[:],
                        rhs=gk[:, 2 * f2 : 2 * f2 + 2, :].rearrange(
                            "p f i -> p (f i)"
                        ),
                        start=(f2 == 0), stop=(f2 == GFG // 2 - 1),
                    )
                # score = ps1_even + ps1_odd + ps2  (one psum operand per op)
                sc = scores_row[:, k * CH : (k + 1) * CH]
                c1 = st.tile([1, 2 * CH], f32, name=f"c1_{k}")
                nc.vector.tensor_copy(out=c1[:], in_=ps1[:])
                nc.vector.tensor_tensor(
                    out=sc, in0=c1[:, 0:CH], in1=c1[:, CH : 2 * CH], op=Alu.add
                )
                nc.vector.tensor_tensor(
                    out=sc, in0=sc, in1=ps2_k[k][:], op=Alu.add
                )

            emit_pass2(0)
            for k in range(1, NCH):
                emit_pass2(k)
                emit_prod(k - 1)
                emit_pass1(k - 1)
            emit_prod(NCH - 1)
            emit_pass1(NCH - 1)

            # ---- finish: sigmoid * 5 -> out slice ----
            nc.scalar.activation(
                out=scores_row[:], in_=scores_row[:], func=Act.Sigmoid
            )
            nc.vector.tensor_scalar_mul(
                out=scores_row[:], in0=scores_row[:], scalar1=5.0
            )
            nc.sync.dma_start(
                out=out_t[:].rearrange("(o n) -> o n", o=1), in_=scores_row[:]
            )

    nc.compile()
    return nc


def make_ubf_in_maps(cfg, A):
    UL = cfg.ul
    return [
        {"ratt_slice": np.ascontiguousarray(
            A[:, k * (UL // 2) : (k + 1) * (UL // 2)])}
        for k in range(cfg.n_cores)
    ]


def make_tables(cfg, rating_mtx, user_similarity, user_bias, item_bias, global_bias):
    import ml_dtypes

    U, I = cfg.n_users, cfg.n_items
    sim = np.asarray(user_similarity, dtype=np.float32)
    R = np.asarray(rating_mtx, dtype=np.float32)
    ub = np.asarray(user_bias, dtype=np.float32)
    ibg = np.asarray(item_bias, dtype=np.float32) + np.float32(np.asarray(global_bias))

    def hilo(x):
        hi = x.astype(ml_dtypes.bfloat16)
        lo = (x - hi.astype(np.float32)).astype(ml_dtypes.bfloat16)
        return hi, lo

    ub_hi, ub_lo = hilo(ub)
    ib_hi, ib_lo = hilo(ibg)

    # G: pair-permuted columns (matches the fp8 16-bit transpose-gather
    # interleave), split into two contiguous halves + a bias table
    c = np.arange(U)
    perm = 2 * ((c // 256) * 128 + (c % 128)) + ((c // 128) % 2)
    Gp = sim[:, perm].astype(ml_dtypes.bfloat16)
    g_lo = np.ascontiguousarray(Gp[:, : U // 2])
    g_hi = np.ascontiguousarray(Gp[:, U // 2 :])
    g_bias = np.zeros((U, 128), ml_dtypes.bfloat16)
    g_bias[:, 0] = ub_hi
    g_bias[:, 1] = ub_lo

    # A: (R.T - 2.5) packed fp8 pairs in i16 units + bf16 bias bit patterns
    A8 = np.ascontiguousarray((R.T - 2.5).astype(ml_dtypes.float8_e4m3fn))
    A = np.zeros((I, cfg.awu), np.int16)
    A[:, : U // 2] = A8.view(np.int16)
    A[:, U // 2] = ib_hi.view(np.int16)
    A[:, U // 2 + 1] = ib_lo.view(np.int16)
    return g_lo, g_hi, g_bias, A


def make_main_in_maps(cfg, user, item, g_lo, g_hi, g_bias, A, nubf):
    import ml_dtypes

    U, BL, GFG, AFU = cfg.n_users, cfg.bl, cfg.gfg, cfg.afu
    u_i = np.asarray(user).astype(np.int64)
    i_i = np.asarray(item).astype(np.int64)
    # scatter -(ubf-2.5)[u] to the pair-permuted (p, fg=2*fu+b) layout
    ubf_ct = np.empty((128, GFG), np.float32)
    fu = np.arange(GFG) // 2
    b = np.arange(GFG) % 2
    p = np.arange(128)
    ubf_ct[:, :] = nubf[2 * (fu[None, :] * 128 + p[:, None]) + b[None, :]]
    ubf_ct = ubf_ct.astype(ml_dtypes.bfloat16)

    maps = []
    for k in range(cfg.n_cores):
        us = u_i[k * BL : (k + 1) * BL].astype(np.int16)
        it = i_i[k * BL : (k + 1) * BL].astype(np.int16)
        uidx = np.tile(us.reshape(BL // 16, 16).T, (8, 1))
        iidx = np.tile(it.reshape(BL // 16, 16).T, (8, 1))
        maps.append(
            {"g_lo": g_lo, "g_hi": g_hi, "g_bias": g_bias, "ratt_aug": A,
             "ubf_ct": ubf_ct, "uidx": uidx, "iidx": iidx}
        )
    return maps


_PROGRAM_CACHE = {}


def _get_programs(cfg: Cfg):
    if cfg not in _PROGRAM_CACHE:
        _PROGRAM_CACHE[cfg] = (build_ubf_program(cfg), build_main_program(cfg))
    return _PROGRAM_CACHE[cfg]


def kernel(user, item, rating_mtx, user_similarity, user_bias, item_bias, global_bias):
    from concourse import bass_utils

    cfg = Cfg()
    assert np.asarray(rating_mtx).shape == (cfg.n_users, cfg.n_items)
    assert np.asarray(user).shape == (cfg.batch,)
    nc_ubf, nc_main = _get_programs(cfg)
    g_lo, g_hi, g_bias, A = make_tables(
        cfg, rating_mtx, user_similarity, user_bias, item_bias, global_bias
    )
    core_ids = list(range(cfg.n_cores))

    res1 = bass_utils.run_bass_kernel_spmd(
        nc_ubf, make_ubf_in_maps(cfg, A), core_ids=core_ids
    )
    # unshard the 8 user-slices of -(ubf-2.5) computed on device
    nubf = np.concatenate(
        [np.asarray(res1.results[k]["ubf"], np.float32).reshape(cfg.ul)
         for k in core_ids]
    )

    res2 = bass_utils.run_bass_kernel_spmd(
        nc_main,
        make_main_in_maps(cfg, user, item, g_lo, g_hi, g_bias, A, nubf),
        core_ids=core_ids,
    )
    out = np.concatenate(
        [np.asarray(res2.results[k]["out"], np.float32) for k in core_ids]
    )
    return out.reshape(cfg.batch)


# revision 23
# speedup vs baseline: 1.2116x; 1.2116x over previous
"""Trainium2 Bass kernel: collaborative-filtering score (segment_reduce).

Math (per batch element b):
    ubf[u]   = masked mean over nonzero entries of rating_mtx[u, :]
    score[b] = sum_u S[user_b, u] * (R[u, item_b] - ubf[u])
    out[b]   = 5 * sigmoid(score[b] + user_bias[user_b] + item_bias[item_b] + gb)

Distribution: BATCH-sharded, TWO launches.

Launch 1 (ubf): user axis sharded 8-way; each core scans its private
[4096 items x 1024 users] fp8 slice of (R.T - 2.5) and produces
-(ubf - 2.5) for its users via PE masked sum/count in fp32 PSUM. The 8
x [1024] slices are concatenated and re-laid-out on the HOST (pure
unshard/reshard of 32KB). A single-launch on-device exchange was built
and measured first: the collectives stack costs ~110us for the
auto-inserted kernel-entry barrier plus ~70us per op on this 8-core
mesh, so any collective-dependent compute cannot start before ~200us --
slower than this entire kernel.

Launch 2 (main): batch sharded 8-way. Core k owns batch slice
[k*1024,(k+1)*1024) and transpose-gathers FULL 8192-user rows per
(user_b, item_b) pair: G (bf16, three contiguous tables: the gather
ucode requires rows <=16KB and contiguous elem_step) and A (fp8 pairs
packed in int16, exact values +-{0.5,1.5,2.5}). Per 256-idx chunk, one
fused fp32-PSUM reduction accumulates [ubf-weighted G terms + hi/lo
bias columns + ones-weighted A(x)G products]; the product pass runs on
DVE (mixed fp8*bf16, single bf16 rounding) in place over gk after the
ubf pass has consumed it. sigmoid*5, DMA out the [1024] slice; the host
concatenates the 8 slices. Only 2048+few gather descriptors per core
(vs 16384 user-sharded), so GPSIMD descriptor generation is off the
critical path and the gathers stream at ~300GB/s.
"""

import sys
from dataclasses import dataclass

import numpy as np

if "/opt/trn_rl_repo" not in sys.path:
    sys.path.insert(0, "/opt/trn_rl_repo")


@dataclass(frozen=True)
class Cfg:
    n_users: int = 8192
    n_items: int = 4096
    batch: int = 8192
    n_cores: int = 8
    ch: int = 256  # gather idxs per chunk

    @property
    def bl(self) -> int:  # batch per core
        return self.batch // self.n_cores

    @property
    def ul(self) -> int:  # users per core (ubf launch)
        return self.n_users // self.n_cores

    @property
    def gfg(self) -> int:  # G data f-groups
        return self.n_users // 128

    @property
    def afu(self) -> int:  # A data unit f-groups (i16 units)
        return self.n_users // 256

    @property
    def awu(self) -> int:  # A row width (i16 units): data + 128-unit bias blk
        return self.n_users // 2 + 128


def build_ubf_program(cfg: Cfg):
    from concourse import bacc, mybir, tile

    f32 = mybir.dt.float32
    i16 = mybir.dt.int16
    fp8 = mybir.dt.float8e4
    Alu = mybir.AluOpType

    I, UL = cfg.n_items, cfg.ul
    ICH = I // 128
    USL = UL // 2  # slice width in i16 units

    nc = bacc.Bacc(None, target_bir_lowering=False, debug=False)
    rsl_t = nc.dram_tensor("ratt_slice", [I, USL], i16, kind="ExternalInput")
    # [128, UL//128] layout: (p, j) holds -(ubf-2.5) for local user j*128+p
    out_t = nc.dram_tensor("ubf", [128, UL // 128], f32, kind="ExternalOutput")

    with tile.TileContext(nc) as tc:
        with (
            tc.tile_pool(name="static", bufs=1) as st,
            tc.tile_pool(name="rstream", bufs=4) as rpool,
            tc.tile_pool(name="maskp", bufs=2) as mpool,
            tc.tile_pool(name="ps", bufs=1, space="PSUM") as pp,
        ):
            ones8 = st.tile([128, 1], fp8)
            nc.gpsimd.memset(ones8[:], 1.0)
            ps_s = [pp.tile([1, 512], f32, name=f"ps_s{j}") for j in range(2)]
            ps_c = [pp.tile([1, 512], f32, name=f"ps_c{j}") for j in range(2)]
            for g in range(ICH):
                rt = rpool.tile([128, USL], i16, name="rt")
                nc.sync.dma_start(out=rt[:], in_=rsl_t[g * 128 : (g + 1) * 128, :])
                atf = rt[:].bitcast(fp8)  # [128, 1024] users in natural order
                mk = mpool.tile([128, UL], fp8, name="mk")
                nc.vector.tensor_scalar(
                    out=mk[:], in0=atf, scalar1=-2.5, scalar2=None,
                    op0=Alu.not_equal,
                )
                first, last = g == 0, g == ICH - 1
                for j in range(2):
                    nc.tensor.matmul(
                        out=ps_s[j][:], lhsT=ones8[:],
                        rhs=atf[:, j * 512 : (j + 1) * 512],
                        start=first, stop=last,
                    )
                    nc.tensor.matmul(
                        out=ps_c[j][:], lhsT=ones8[:],
                        rhs=mk[:, j * 512 : (j + 1) * 512],
                        start=first, stop=last,
                    )

            # scatter sums/counts to [128, 8] ((p,j) <- user j*128+p) via
            # K=1 matmuls so the elementwise finish runs 128 lanes wide
            # (a [1,1024] single-partition reciprocal alone costs ~7us).
            sum_row = st.tile([1, UL], f32)
            cnt_row = st.tile([1, UL], f32)
            for j in range(2):
                nc.vector.tensor_copy(
                    out=sum_row[:, j * 512 : (j + 1) * 512], in_=ps_s[j][:]
                )
                nc.vector.tensor_copy(
                    out=cnt_row[:, j * 512 : (j + 1) * 512], in_=ps_c[j][:]
                )
            one1 = st.tile([1, 1], f32)
            nc.gpsimd.memset(one1[:], 1.0)
            JC = UL // 128
            ps_sc = pp.tile([128, JC], f32, name="ps_sc", tag="ps_s0")
            ps_cc = pp.tile([128, JC], f32, name="ps_cc", tag="ps_c0")
            for j in range(JC):
                nc.tensor.matmul(
                    out=ps_sc[:, j : j + 1],
                    lhsT=sum_row[:, j * 128 : (j + 1) * 128],
                    rhs=one1[:], start=True, stop=True,
                )
                nc.tensor.matmul(
                    out=ps_cc[:, j : j + 1],
                    lhsT=cnt_row[:, j * 128 : (j + 1) * 128],
                    rhs=one1[:], start=True, stop=True,
                )
            # table is pre-centered: sum_A = sum(R) - 2.5*I, so
            # -(ubf - 2.5) = -((sum_A + 2.5*I)/max(cnt,1) - 2.5); cnt==0
            # rows give +2.5 = -(0 - 2.5), matching the reference's ubf=0.
            rcp = st.tile([128, JC], f32)
            nc.vector.tensor_scalar(
                out=rcp[:], in0=ps_cc[:], scalar1=1.0, scalar2=None,
                op0=Alu.max,
            )
            nc.vector.reciprocal(out=rcp[:], in_=rcp[:])
            ubf_loc = st.tile([128, JC], f32)
            nc.vector.scalar_tensor_tensor(
                out=ubf_loc[:], in0=ps_sc[:], scalar=2.5 * I, in1=rcp[:],
                op0=Alu.add, op1=Alu.mult,
            )
            nc.vector.tensor_scalar(
                out=ubf_loc[:], in0=ubf_loc[:], scalar1=2.5, scalar2=-1.0,
                op0=Alu.subtract, op1=Alu.mult,
            )
            nc.sync.dma_start(out=out_t[:], in_=ubf_loc[:])

    nc.compile()
    return nc


def build_main_program(cfg: Cfg):
    from concourse import bacc, mybir, tile

    f32 = mybir.dt.float32
    i16 = mybir.dt.int16
    bf16 = mybir.dt.bfloat16
    fp8 = mybir.dt.float8e4
    Alu = mybir.AluOpType
    Act = mybir.ActivationFunctionType

    U, I, BL, CH = cfg.n_users, cfg.n_items, cfg.bl, cfg.ch
    GFG, AFU, AWU = cfg.gfg, cfg.afu, cfg.awu
    NCH = BL // CH
    IDXC = BL // 16

    nc = bacc.Bacc(
        None, target_bir_lowering=False, debug=False, num_swdge_queues=2
    )

    glo_t = nc.dram_tensor("g_lo", [U, U // 2], bf16, kind="ExternalInput")
    ghi_t = nc.dram_tensor("g_hi", [U, U // 2], bf16, kind="ExternalInput")
    gbi_t = nc.dram_tensor("g_bias", [U, 128], bf16, kind="ExternalInput")
    rtt_t = nc.dram_tensor("ratt_aug", [I, AWU], i16, kind="ExternalInput")
    ubfc_t = nc.dram_tensor("ubf_ct", [128, GFG], bf16, kind="ExternalInput")
    uidx_t = nc.dram_tensor("uidx", [128, IDXC], i16, kind="ExternalInput")
    iidx_t = nc.dram_tensor("iidx", [128, IDXC], i16, kind="ExternalInput")
    out_t = nc.dram_tensor("out", [BL], f32, kind="ExternalOutput")

    with tile.TileContext(nc) as tc:
        with (
            tc.tile_pool(name="static", bufs=1) as st,
            tc.tile_pool(name="gpool", bufs=3) as gpool,
            tc.tile_pool(name="apool", bufs=NCH) as apool,
            tc.tile_pool(name="cpool", bufs=2) as cpool,
            tc.tile_pool(name="psum", bufs=2, space="PSUM") as pp,
        ):
            # ---- statics ----
            ones_b = st.tile([128, 1], bf16)
            nc.gpsimd.memset(ones_b[:], 1.0)
            uidx_sb = st.tile([128, IDXC], i16)
            nc.sync.dma_start(out=uidx_sb[:], in_=uidx_t[:])
            iidx_sb = st.tile([128, IDXC], i16)
            nc.sync.dma_start(out=iidx_sb[:], in_=iidx_t[:])
            ubf_ct = st.tile([128, GFG], bf16)
            nc.sync.dma_start(out=ubf_ct[:], in_=ubfc_t[:])

            # ---- gathers: interleaved per chunk so A data lands with its G
            # counterpart (descriptor generation is serial on the gpsimd
            # engine; queue-0 ring-space waits pace it to the DMA rate, so
            # emitting all G first starves A until ~100us). apool holds all
            # chunks so A generation never blocks the queue. ----
            gks, aks = [], []
            icn = CH // 16
            H = GFG // 2
            for k in range(NCH):
                gk = gpool.tile([128, GFG + 1, CH], bf16, name="gk")
                ak = apool.tile([128, AFU + 1, CH], i16, name="ak")
                idx = uidx_sb[:, k * icn : (k + 1) * icn]
                nc.gpsimd.dma_gather(
                    out_ap=gk[:, 0:H, :], in_ap=glo_t[:], idxs_ap=idx,
                    num_idxs=CH, num_idxs_reg=CH, elem_size=U // 2,
                    transpose=True, queue_num=0,
                )
                nc.gpsimd.dma_gather(
                    out_ap=ak[:], in_ap=rtt_t[:],
                    idxs_ap=iidx_sb[:, k * icn : (k + 1) * icn],
                    num_idxs=CH, num_idxs_reg=CH, elem_size=AWU,
                    transpose=True, queue_num=1,
                )
                nc.gpsimd.dma_gather(
                    out_ap=gk[:, H : 2 * H, :], in_ap=ghi_t[:], idxs_ap=idx,
                    num_idxs=CH, num_idxs_reg=CH, elem_size=U // 2,
                    transpose=True, queue_num=0,
                )
                nc.gpsimd.dma_gather(
                    out_ap=gk[:, GFG : GFG + 1, :], in_ap=gbi_t[:], idxs_ap=idx,
                    num_idxs=CH, num_idxs_reg=CH, elem_size=128,
                    transpose=True, queue_num=0,
                )
                gks.append(gk)
                aks.append(ak)

            # ---- main loop ----
            # per chunk: ps2[1,CH] += sum_u -ubf_c[u]*G + bias columns (PE,
            # reads raw gk); then p = A(x)G overwrites gk in place (DVE);
            # then ps1[1,2*CH] += fg-pair-merged product sums (PE). Emission
            # is software-pipelined so P2(k+1) runs while DVE computes p(k).
            scores_row = st.tile([1, BL], f32)
            ps1_k, ps2_k = [None] * NCH, [None] * NCH

            def emit_pass2(k):
                # M=2 trick: lhsT [128,2] = [w_even | w_odd], rhs = the fg
                # pair flattened [128, 2*CH]; the diagonal blocks
                # out[0, 0:CH] and out[1, CH:2CH] hold the wanted sums (the
                # cross blocks accumulate harmlessly). Halves the matmul
                # count vs one mm per f-group.
                gk, ak = gks[k], aks[k]
                ps2 = pp.tile([2, 2 * CH], f32, name="ps2")
                ps2_k[k] = ps2
                for f2 in range(GFG // 2):
                    nc.tensor.matmul(
                        out=ps2[:], lhsT=ubf_ct[:, 2 * f2 : 2 * f2 + 2],
                        rhs=gk[:, 2 * f2 : 2 * f2 + 2, :].rearrange(
                            "p f i -> p (f i)"
                        ),
                        start=(f2 == 0), stop=False,
                    )
                nc.tensor.matmul(
                    out=ps2[0:1, 0:CH], lhsT=ones_b[:], rhs=gk[:, GFG, :],
                    start=False, stop=False,
                )
                nc.tensor.matmul(
                    out=ps2[0:1, 0:CH], lhsT=ones_b[:],
                    rhs=ak[:, AFU, :].bitcast(bf16),
                    start=False, stop=True,
                )

            def emit_prod(k):
                gk, ak = gks[k], aks[k]
                akf = ak[:, 0:AFU, :].bitcast(fp8).rearrange(
                    "p f (i b) -> p f b i", b=2
                )
                gkv = gk[:, 0:GFG, :].rearrange("p (f b) i -> p f b i", b=2)
                nc.vector.tensor_tensor(out=gkv, in0=akf, in1=gkv, op=Alu.mult)

            def emit_pass1(k):
                gk = gks[k]
                ps1 = pp.tile([1, 2 * CH], f32, name="ps1")
                ps1_k[k] = ps1
                for f2 in range(GFG // 2):
                    nc.tensor.matmul(
                        out=ps1[:],
                        lhsT=ones_b[:],
                        rhs=gk[:, 2 * f2 : 2 * f2 + 2, :].rearrange(
                            "p f i -> p (f i)"
                        ),
                        start=(f2 == 0), stop=(f2 == GFG // 2 - 1),
                    )
                # score = ps1_even + ps1_odd + ps2[0,even] + ps2[1,odd].
                # Engines cannot start an access at partition 1, so bounce
                # ps2's row-1 block to partition 0 with a tiny SBUF DMA.
                sc = scores_row[:, k * CH : (k + 1) * CH]
                c1 = cpool.tile([1, 2 * CH], f32, name="c1")
                nc.vector.tensor_copy(out=c1[:], in_=ps1[:])
                c2 = cpool.tile([2, 2 * CH], f32, name="c2")
                nc.vector.tensor_copy(out=c2[:], in_=ps2_k[k][:])
                c2b = cpool.tile([1, CH], f32, name="c2b")
                nc.sync.dma_start(out=c2b[:], in_=c2[1:2, CH : 2 * CH])
                nc.vector.tensor_tensor(
                    out=sc, in0=c1[:, 0:CH], in1=c1[:, CH : 2 * CH], op=Alu.add
                )
                nc.vector.tensor_tensor(
                    out=sc, in0=sc, in1=c2[0:1, 0:CH], op=Alu.add
                )
                nc.vector.tensor_tensor(
                    out=sc, in0=sc, in1=c2b[:], op=Alu.add
                )

            emit_pass2(0)
            for k in range(1, NCH):
                emit_pass2(k)
                emit_prod(k - 1)
                emit_pass1(k - 1)
            emit_prod(NCH - 1)
            emit_pass1(NCH - 1)

            # ---- finish: sigmoid * 5 -> out slice ----
            nc.scalar.activation(
                out=scores_row[:], in_=scores_row[:], func=Act.Sigmoid
            )
            nc.vector.tensor_scalar_mul(
                out=scores_row[:], in0=scores_row[:], scalar1=5.0
            )
            nc.sync.dma_start(
                out=out_t[:].rearrange("(o n) -> o n", o=1), in_=scores_row[:]
            )

    nc.compile()
    return nc


def make_ubf_in_maps(cfg, A):
    UL = cfg.ul
    return [
        {"ratt_slice": np.ascontiguousarray(
            A[:, k * (UL // 2) : (k + 1) * (UL // 2)])}
        for k in range(cfg.n_cores)
    ]


def make_tables(cfg, rating_mtx, user_similarity, user_bias, item_bias, global_bias):
    import ml_dtypes

    U, I = cfg.n_users, cfg.n_items
    sim = np.asarray(user_similarity, dtype=np.float32)
    R = np.asarray(rating_mtx, dtype=np.float32)
    ub = np.asarray(user_bias, dtype=np.float32)
    ibg = np.asarray(item_bias, dtype=np.float32) + np.float32(np.asarray(global_bias))

    def hilo(x):
        hi = x.astype(ml_dtypes.bfloat16)
        lo = (x - hi.astype(np.float32)).astype(ml_dtypes.bfloat16)
        return hi, lo

    ub_hi, ub_lo = hilo(ub)
    ib_hi, ib_lo = hilo(ibg)

    # G: pair-permuted columns (matches the fp8 16-bit transpose-gather
    # interleave), split into two contiguous halves + a bias table
    c = np.arange(U)
    perm = 2 * ((c // 256) * 128 + (c % 128)) + ((c // 128) % 2)
    Gp = sim[:, perm].astype(ml_dtypes.bfloat16)
    g_lo = np.ascontiguousarray(Gp[:, : U // 2])
    g_hi = np.ascontiguousarray(Gp[:, U // 2 :])
    g_bias = np.zeros((U, 128), ml_dtypes.bfloat16)
    g_bias[:, 0] = ub_hi
    g_bias[:, 1] = ub_lo

    # A: (R.T - 2.5) packed fp8 pairs in i16 units + bf16 bias bit patterns
    A8 = np.ascontiguousarray((R.T - 2.5).astype(ml_dtypes.float8_e4m3fn))
    A = np.zeros((I, cfg.awu), np.int16)
    A[:, : U // 2] = A8.view(np.int16)
    A[:, U // 2] = ib_hi.view(np.int16)
    A[:, U // 2 + 1] = ib_lo.view(np.int16)
    return g_lo, g_hi, g_bias, A


def make_main_in_maps(cfg, user, item, g_lo, g_hi, g_bias, A, nubf):
    import ml_dtypes

    U, BL, GFG, AFU = cfg.n_users, cfg.bl, cfg.gfg, cfg.afu
    u_i = np.asarray(user).astype(np.int64)
    i_i = np.asarray(item).astype(np.int64)
    # scatter -(ubf-2.5)[u] to the pair-permuted (p, fg=2*fu+b) layout
    ubf_ct = np.empty((128, GFG), np.float32)
    fu = np.arange(GFG) // 2
    b = np.arange(GFG) % 2
    p = np.arange(128)
    ubf_ct[:, :] = nubf[2 * (fu[None, :] * 128 + p[:, None]) + b[None, :]]
    ubf_ct = ubf_ct.astype(ml_dtypes.bfloat16)

    maps = []
    for k in range(cfg.n_cores):
        us = u_i[k * BL : (k + 1) * BL].astype(np.int16)
        it = i_i[k * BL : (k + 1) * BL].astype(np.int16)
        uidx = np.tile(us.reshape(BL // 16, 16).T, (8, 1))
        iidx = np.tile(it.reshape(BL // 16, 16).T, (8, 1))
        maps.append(
            {"g_lo": g_lo, "g_hi": g_hi, "g_bias": g_bias, "ratt_aug": A,
             "ubf_ct": ubf_ct, "uidx": uidx, "iidx": iidx}
        )
    return maps


_PROGRAM_CACHE = {}


def _get_programs(cfg: Cfg):
    if cfg not in _PROGRAM_CACHE:
        _PROGRAM_CACHE[cfg] = (build_ubf_program(cfg), build_main_program(cfg))
    return _PROGRAM_CACHE[cfg]


def kernel(user, item, rating_mtx, user_similarity, user_bias, item_bias, global_bias):
    from concourse import bass_utils

    cfg = Cfg()
    assert np.asarray(rating_mtx).shape == (cfg.n_users, cfg.n_items)
    assert np.asarray(user).shape == (cfg.batch,)
    nc_ubf, nc_main = _get_programs(cfg)
    g_lo, g_hi, g_bias, A = make_tables(
        cfg, rating_mtx, user_similarity, user_bias, item_bias, global_bias
    )
    core_ids = list(range(cfg.n_cores))

    res1 = bass_utils.run_bass_kernel_spmd(
        nc_ubf, make_ubf_in_maps(cfg, A), core_ids=core_ids
    )
    # unshard the 8 user-slices of -(ubf-2.5) computed on device
    # (device layout [128, 8]: (p, j) = local user j*128+p)
    nubf = np.concatenate(
        [np.asarray(res1.results[k]["ubf"], np.float32)
         .reshape(128, cfg.ul // 128).T.ravel()
         for k in core_ids]
    )

    res2 = bass_utils.run_bass_kernel_spmd(
        nc_main,
        make_main_in_maps(cfg, user, item, g_lo, g_hi, g_bias, A, nubf),
        core_ids=core_ids,
    )
    out = np.concatenate(
        [np.asarray(res2.results[k]["out"], np.float32) for k in core_ids]
    )
    return out.reshape(cfg.batch)


# revision 34
# speedup vs baseline: 1.3050x; 1.0770x over previous
"""Trainium2 Bass kernel: collaborative-filtering score (segment_reduce).

Math (per batch element b):
    ubf[u]   = masked mean over nonzero entries of rating_mtx[u, :]
    score[b] = sum_u S[user_b, u] * (R[u, item_b] - ubf[u])
    out[b]   = 5 * sigmoid(score[b] + user_bias[user_b] + item_bias[item_b] + gb)

Distribution: BATCH-sharded, TWO launches.

Launch 1 (ubf): user axis sharded 8-way; each core scans its private
[4096 items x 1024 users] fp8 slice of (R.T - 2.5) and produces
-(ubf - 2.5) for its users via PE masked sum/count in fp32 PSUM. The 8
x [1024] slices are concatenated and re-laid-out on the HOST (pure
unshard/reshard of 32KB). A single-launch on-device exchange was built
and measured first: the collectives stack costs ~110us for the
auto-inserted kernel-entry barrier plus ~70us per op on this 8-core
mesh, so any collective-dependent compute cannot start before ~200us --
slower than this entire kernel.

Launch 2 (main): batch sharded 8-way. Core k owns batch slice
[k*1024,(k+1)*1024) and transpose-gathers FULL 8192-user rows per
(user_b, item_b) pair: G (bf16, three contiguous tables: the gather
ucode requires rows <=16KB and contiguous elem_step) and A (fp8 pairs
packed in int16, exact values +-{0.5,1.5,2.5}). Per 256-idx chunk, one
fused fp32-PSUM reduction accumulates [ubf-weighted G terms + hi/lo
bias columns + ones-weighted A(x)G products]; the product pass runs on
DVE (mixed fp8*bf16, single bf16 rounding) in place over gk after the
ubf pass has consumed it. sigmoid*5, DMA out the [1024] slice; the host
concatenates the 8 slices. Only 2048+few gather descriptors per core
(vs 16384 user-sharded), so GPSIMD descriptor generation is off the
critical path and the gathers stream at ~300GB/s.
"""

import sys
from dataclasses import dataclass

import numpy as np

if "/opt/trn_rl_repo" not in sys.path:
    sys.path.insert(0, "/opt/trn_rl_repo")


@dataclass(frozen=True)
class Cfg:
    n_users: int = 8192
    n_items: int = 4096
    batch: int = 8192
    n_cores: int = 8
    ch: int = 256  # gather idxs per chunk

    @property
    def bl(self) -> int:  # batch per core
        return self.batch // self.n_cores

    @property
    def ul(self) -> int:  # users per core (ubf launch)
        return self.n_users // self.n_cores

    @property
    def gfg(self) -> int:  # G data f-groups
        return self.n_users // 128

    @property
    def afu(self) -> int:  # A data unit f-groups (i16 units)
        return self.n_users // 256

    @property
    def awu(self) -> int:  # A row width (i16 units): data + 128-unit bias blk
        return self.n_users // 2 + 128


def build_ubf_program(cfg: Cfg):
    from concourse import bacc, mybir, tile

    f32 = mybir.dt.float32
    i16 = mybir.dt.int16
    fp8 = mybir.dt.float8e4
    Alu = mybir.AluOpType

    I, UL = cfg.n_items, cfg.ul
    USL = UL // 2  # slice width in i16 units
    NSUB = 4  # load sub-tiles (pipelining granularity)
    RT = I // 128  # item-rows per partition (32)
    RS = RT // NSUB  # item-rows per partition per sub-tile

    nc = bacc.Bacc(None, target_bir_lowering=False, debug=False)
    rsl_t = nc.dram_tensor("ratt_slice", [I, USL], i16, kind="ExternalInput")
    # [128, UL//128] layout: (p, j) holds -(ubf-2.5) for local user j*128+p
    out_t = nc.dram_tensor("ubf", [128, UL // 128], f32, kind="ExternalOutput")

    with tile.TileContext(nc) as tc:
        with (
            tc.tile_pool(name="static", bufs=1) as st,
            tc.tile_pool(name="rstream", bufs=2) as rpool,
            tc.tile_pool(name="maskp", bufs=2) as mpool,
            tc.tile_pool(name="ps", bufs=1, space="PSUM") as pp,
        ):
            # partition p of sub-tile s holds item rows (p*RT + s*RS ..
            # + RS): per-partition contiguous 8KB reads, full HBM bandwidth.
            rsl_v = rsl_t[:].rearrange("(p r) u -> p r u", p=128)
            # dual-fp8 ldweights wants the k-tile pair 16-element strided
            ones8 = st.tile([128, 2, 16], fp8)
            nc.gpsimd.memset(ones8[:], 1.0)
            ps_s = [pp.tile([1, 512], f32, name=f"ps_s{j}") for j in range(2)]
            ps_c = [pp.tile([1, 512], f32, name=f"ps_c{j}") for j in range(2)]
            DR = mybir.MatmulPerfMode.DoubleRow
            for s in range(NSUB):
                rt = rpool.tile([128, RS, USL], i16, name="rt")
                nc.sync.dma_start(
                    out=rt[:], in_=rsl_v[:, s * RS : (s + 1) * RS, :]
                )
                atf = rt[:].bitcast(fp8)  # [128, RS, 1024] users natural
                mk = mpool.tile([128, RS, UL], fp8, name="mk")
                nc.vector.tensor_scalar(
                    out=mk[:], in0=atf, scalar1=-2.5, scalar2=None,
                    op0=Alu.not_equal,
                )
                first, last = s == 0, s == NSUB - 1
                for r2 in range(RS // 2):
                    fr, lr = first and r2 == 0, last and r2 == RS // 2 - 1
                    for j in range(2):
                        nc.tensor.matmul(
                            out=ps_s[j][:], lhsT=ones8[:, :, 0:1],
                            rhs=atf[:, 2 * r2 : 2 * r2 + 2, j * 512 : (j + 1) * 512],
                            start=fr, stop=lr, perf_mode=DR,
                        )
                        nc.tensor.matmul(
                            out=ps_c[j][:], lhsT=ones8[:, :, 0:1],
                            rhs=mk[:, 2 * r2 : 2 * r2 + 2, j * 512 : (j + 1) * 512],
                            start=fr, stop=lr, perf_mode=DR,
                        )

            # scatter sums/counts to [128, 8] ((p,j) <- user j*128+p) via
            # K=1 matmuls so the elementwise finish runs 128 lanes wide
            # (a [1,1024] single-partition reciprocal alone costs ~7us).
            sum_row = st.tile([1, UL], f32)
            cnt_row = st.tile([1, UL], f32)
            for j in range(2):
                nc.vector.tensor_copy(
                    out=sum_row[:, j * 512 : (j + 1) * 512], in_=ps_s[j][:]
                )
                nc.vector.tensor_copy(
                    out=cnt_row[:, j * 512 : (j + 1) * 512], in_=ps_c[j][:]
                )
            one1 = st.tile([1, 1], f32)
            nc.gpsimd.memset(one1[:], 1.0)
            JC = UL // 128
            ps_sc = pp.tile([128, JC], f32, name="ps_sc", tag="ps_s0")
            ps_cc = pp.tile([128, JC], f32, name="ps_cc", tag="ps_c0")
            for j in range(JC):
                nc.tensor.matmul(
                    out=ps_sc[:, j : j + 1],
                    lhsT=sum_row[:, j * 128 : (j + 1) * 128],
                    rhs=one1[:], start=True, stop=True,
                )
                nc.tensor.matmul(
                    out=ps_cc[:, j : j + 1],
                    lhsT=cnt_row[:, j * 128 : (j + 1) * 128],
                    rhs=one1[:], start=True, stop=True,
                )
            # table is pre-centered: sum_A = sum(R) - 2.5*I, so
            # -(ubf - 2.5) = -((sum_A + 2.5*I)/max(cnt,1) - 2.5); cnt==0
            # rows give +2.5 = -(0 - 2.5), matching the reference's ubf=0.
            rcp = st.tile([128, JC], f32)
            nc.vector.tensor_scalar(
                out=rcp[:], in0=ps_cc[:], scalar1=1.0, scalar2=None,
                op0=Alu.max,
            )
            nc.vector.reciprocal(out=rcp[:], in_=rcp[:])
            ubf_loc = st.tile([128, JC], f32)
            nc.vector.scalar_tensor_tensor(
                out=ubf_loc[:], in0=ps_sc[:], scalar=2.5 * I, in1=rcp[:],
                op0=Alu.add, op1=Alu.mult,
            )
            nc.vector.tensor_scalar(
                out=ubf_loc[:], in0=ubf_loc[:], scalar1=2.5, scalar2=-1.0,
                op0=Alu.subtract, op1=Alu.mult,
            )
            nc.sync.dma_start(out=out_t[:], in_=ubf_loc[:])

    nc.compile()
    return nc


def build_main_program(cfg: Cfg):
    from concourse import bacc, mybir, tile

    f32 = mybir.dt.float32
    i16 = mybir.dt.int16
    bf16 = mybir.dt.bfloat16
    fp8 = mybir.dt.float8e4
    Alu = mybir.AluOpType
    Act = mybir.ActivationFunctionType

    U, I, BL, CH = cfg.n_users, cfg.n_items, cfg.bl, cfg.ch
    GFG, AFU, AWU = cfg.gfg, cfg.afu, cfg.awu
    NCH = BL // CH
    IDXC = BL // 16

    nc = bacc.Bacc(
        None, target_bir_lowering=False, debug=False, num_swdge_queues=2
    )

    glo_t = nc.dram_tensor("g_lo", [U, U // 2], bf16, kind="ExternalInput")
    ghi_t = nc.dram_tensor("g_hi", [U, U // 2], bf16, kind="ExternalInput")
    rtt_t = nc.dram_tensor("ratt_aug", [I, AWU], i16, kind="ExternalInput")
    ubb_t = nc.dram_tensor("ubb", [1, BL], f32, kind="ExternalInput")
    ubfc_t = nc.dram_tensor("ubf_ct", [128, GFG], bf16, kind="ExternalInput")
    uidx_t = nc.dram_tensor("uidx", [128, IDXC], i16, kind="ExternalInput")
    iidx_t = nc.dram_tensor("iidx", [128, IDXC], i16, kind="ExternalInput")
    out_t = nc.dram_tensor("out", [BL], f32, kind="ExternalOutput")

    with tile.TileContext(nc) as tc:
        with (
            tc.tile_pool(name="static", bufs=1) as st,
            tc.tile_pool(name="gpool", bufs=3) as gpool,
            tc.tile_pool(name="apool", bufs=NCH) as apool,
            tc.tile_pool(name="cpool", bufs=2) as cpool,
            tc.tile_pool(name="psum", bufs=2, space="PSUM") as pp,
        ):
            # ---- statics ----
            ones_b = st.tile([128, 1], bf16)
            nc.gpsimd.memset(ones_b[:], 1.0)
            uidx_sb = st.tile([128, IDXC], i16)
            nc.sync.dma_start(out=uidx_sb[:], in_=uidx_t[:])
            iidx_sb = st.tile([128, IDXC], i16)
            nc.sync.dma_start(out=iidx_sb[:], in_=iidx_t[:])
            ubf_ct = st.tile([128, GFG], bf16)
            nc.sync.dma_start(out=ubf_ct[:], in_=ubfc_t[:])
            ubb_sb = st.tile([1, BL], f32)
            nc.sync.dma_start(out=ubb_sb[:], in_=ubb_t[:])

            # ---- gathers: interleaved per chunk so A data lands with its G
            # counterpart (descriptor generation is serial on the gpsimd
            # engine; queue-0 ring-space waits pace it to the DMA rate, so
            # emitting all G first starves A until ~100us). apool holds all
            # chunks so A generation never blocks the queue. ----
            gks, aks = [], []
            icn = CH // 16
            H = GFG // 2
            for k in range(NCH):
                gk = gpool.tile([128, GFG, CH], bf16, name="gk")
                ak = apool.tile([128, AFU + 1, CH], i16, name="ak")
                idx = uidx_sb[:, k * icn : (k + 1) * icn]
                nc.gpsimd.dma_gather(
                    out_ap=gk[:, 0:H, :], in_ap=glo_t[:], idxs_ap=idx,
                    num_idxs=CH, num_idxs_reg=CH, elem_size=U // 2,
                    transpose=True, queue_num=0,
                )
                nc.gpsimd.dma_gather(
                    out_ap=ak[:], in_ap=rtt_t[:],
                    idxs_ap=iidx_sb[:, k * icn : (k + 1) * icn],
                    num_idxs=CH, num_idxs_reg=CH, elem_size=AWU,
                    transpose=True, queue_num=1,
                )
                nc.gpsimd.dma_gather(
                    out_ap=gk[:, H : 2 * H, :], in_ap=ghi_t[:], idxs_ap=idx,
                    num_idxs=CH, num_idxs_reg=CH, elem_size=U // 2,
                    transpose=True, queue_num=0,
                )
                gks.append(gk)
                aks.append(ak)

            # ---- main loop ----
            # per chunk: ps2[1,CH] += sum_u -ubf_c[u]*G + bias columns (PE,
            # reads raw gk); then p = A(x)G overwrites gk in place (DVE);
            # then ps1[1,2*CH] += fg-pair-merged product sums (PE). Emission
            # is software-pipelined so P2(k+1) runs while DVE computes p(k).
            scores_row = st.tile([1, BL], f32)
            ps1_k, ps2_k = [None] * NCH, [None] * NCH

            def emit_pass2(k):
                # M=2 trick: lhsT [128,2] = [w_even | w_odd], rhs = the fg
                # pair flattened [128, 2*CH]; the diagonal blocks
                # out[0, 0:CH] and out[1, CH:2CH] hold the wanted sums (the
                # cross blocks accumulate harmlessly). Halves the matmul
                # count vs one mm per f-group.
                gk, ak = gks[k], aks[k]
                ps2 = pp.tile([2, 2 * CH], f32, name="ps2")
                ps2_k[k] = ps2
                for f2 in range(GFG // 2):
                    nc.tensor.matmul(
                        out=ps2[:], lhsT=ubf_ct[:, 2 * f2 : 2 * f2 + 2],
                        rhs=gk[:, 2 * f2 : 2 * f2 + 2, :].rearrange(
                            "p f i -> p (f i)"
                        ),
                        start=(f2 == 0), stop=False,
                    )
                nc.tensor.matmul(
                    out=ps2[0:1, 0:CH], lhsT=ones_b[:],
                    rhs=ak[:, AFU, :].bitcast(bf16),
                    start=False, stop=True,
                )

            def emit_prod(k):
                gk, ak = gks[k], aks[k]
                akf = ak[:, 0:AFU, :].bitcast(fp8).rearrange(
                    "p f (i b) -> p f b i", b=2
                )
                gkv = gk[:, 0:GFG, :].rearrange("p (f b) i -> p f b i", b=2)
                nc.vector.tensor_tensor(out=gkv, in0=akf, in1=gkv, op=Alu.mult)

            def emit_pass1(k):
                gk = gks[k]
                ps1 = pp.tile([1, 2 * CH], f32, name="ps1")
                ps1_k[k] = ps1
                for f2 in range(GFG // 2):
                    nc.tensor.matmul(
                        out=ps1[:],
                        lhsT=ones_b[:],
                        rhs=gk[:, 2 * f2 : 2 * f2 + 2, :].rearrange(
                            "p f i -> p (f i)"
                        ),
                        start=(f2 == 0), stop=(f2 == GFG // 2 - 1),
                    )
                # score = ps1_even + ps1_odd + ps2[0,even] + ps2[1,odd].
                # Engines cannot start an access at partition 1, so bounce
                # ps2's row-1 block to partition 0 with a tiny SBUF DMA.
                sc = scores_row[:, k * CH : (k + 1) * CH]
                c1 = cpool.tile([1, 2 * CH], f32, name="c1")
                nc.vector.tensor_copy(out=c1[:], in_=ps1[:])
                c2 = cpool.tile([2, 2 * CH], f32, name="c2")
                nc.vector.tensor_copy(out=c2[:], in_=ps2_k[k][:])
                c2b = cpool.tile([1, CH], f32, name="c2b")
                nc.sync.dma_start(out=c2b[:], in_=c2[1:2, CH : 2 * CH])
                nc.vector.tensor_tensor(
                    out=sc, in0=c1[:, 0:CH], in1=c1[:, CH : 2 * CH], op=Alu.add
                )
                nc.vector.tensor_tensor(
                    out=sc, in0=sc, in1=c2[0:1, 0:CH], op=Alu.add
                )
                nc.vector.tensor_tensor(
                    out=sc, in0=sc, in1=c2b[:], op=Alu.add
                )

            emit_pass2(0)
            for k in range(1, NCH):
                emit_pass2(k)
                emit_prod(k - 1)
                emit_pass1(k - 1)
            emit_prod(NCH - 1)
            emit_pass1(NCH - 1)

            # ---- finish: + user_bias[user_b], sigmoid * 5 -> out slice ----
            nc.vector.tensor_tensor(
                out=scores_row[:], in0=scores_row[:], in1=ubb_sb[:], op=Alu.add
            )
            nc.scalar.activation(
                out=scores_row[:], in_=scores_row[:], func=Act.Sigmoid
            )
            nc.vector.tensor_scalar_mul(
                out=scores_row[:], in0=scores_row[:], scalar1=5.0
            )
            nc.sync.dma_start(
                out=out_t[:].rearrange("(o n) -> o n", o=1), in_=scores_row[:]
            )

    nc.compile()
    return nc


def make_ubf_in_maps(cfg, A):
    UL = cfg.ul
    return [
        {"ratt_slice": np.ascontiguousarray(
            A[:, k * (UL // 2) : (k + 1) * (UL // 2)])}
        for k in range(cfg.n_cores)
    ]


def make_tables(cfg, rating_mtx, user_similarity, user_bias, item_bias, global_bias):
    import ml_dtypes

    U, I = cfg.n_users, cfg.n_items
    sim = np.asarray(user_similarity, dtype=np.float32)
    R = np.asarray(rating_mtx, dtype=np.float32)
    ibg = np.asarray(item_bias, dtype=np.float32) + np.float32(np.asarray(global_bias))

    def hilo(x):
        hi = x.astype(ml_dtypes.bfloat16)
        lo = (x - hi.astype(np.float32)).astype(ml_dtypes.bfloat16)
        return hi, lo

    ib_hi, ib_lo = hilo(ibg)

    # G: pair-permuted columns (matches the fp8 16-bit transpose-gather
    # interleave), split into two contiguous halves (gather rows <16KB)
    c = np.arange(U)
    perm = 2 * ((c // 256) * 128 + (c % 128)) + ((c // 128) % 2)
    Gp = sim[:, perm].astype(ml_dtypes.bfloat16)
    g_lo = np.ascontiguousarray(Gp[:, : U // 2])
    g_hi = np.ascontiguousarray(Gp[:, U // 2 :])

    # A: (R.T - 2.5) packed fp8 pairs in i16 units + bf16 bias bit patterns
    A8 = np.ascontiguousarray((R.T - 2.5).astype(ml_dtypes.float8_e4m3fn))
    A = np.zeros((I, cfg.awu), np.int16)
    A[:, : U // 2] = A8.view(np.int16)
    A[:, U // 2] = ib_hi.view(np.int16)
    A[:, U // 2 + 1] = ib_lo.view(np.int16)
    return g_lo, g_hi, A


def make_main_in_maps(cfg, user, item, g_lo, g_hi, A, nubf, user_bias):
    import ml_dtypes

    U, BL, GFG = cfg.n_users, cfg.bl, cfg.gfg
    u_i = np.asarray(user).astype(np.int64)
    i_i = np.asarray(item).astype(np.int64)
    ub = np.asarray(user_bias, dtype=np.float32)
    # scatter -(ubf-2.5)[u] to the pair-permuted (p, fg=2*fu+b) layout
    ubf_ct = np.empty((128, GFG), np.float32)
    fu = np.arange(GFG) // 2
    b = np.arange(GFG) % 2
    p = np.arange(128)
    ubf_ct[:, :] = nubf[2 * (fu[None, :] * 128 + p[:, None]) + b[None, :]]
    ubf_ct = ubf_ct.astype(ml_dtypes.bfloat16)

    maps = []
    for k in range(cfg.n_cores):
        us = u_i[k * BL : (k + 1) * BL].astype(np.int16)
        it = i_i[k * BL : (k + 1) * BL].astype(np.int16)
        uidx = np.tile(us.reshape(BL // 16, 16).T, (8, 1))
        iidx = np.tile(it.reshape(BL // 16, 16).T, (8, 1))
        ubb = ub[u_i[k * BL : (k + 1) * BL]].reshape(1, BL)
        maps.append(
            {"g_lo": g_lo, "g_hi": g_hi, "ratt_aug": A, "ubb": ubb,
             "ubf_ct": ubf_ct, "uidx": uidx, "iidx": iidx}
        )
    return maps


_PROGRAM_CACHE = {}


def _get_programs(cfg: Cfg):
    if cfg not in _PROGRAM_CACHE:
        _PROGRAM_CACHE[cfg] = (build_ubf_program(cfg), build_main_program(cfg))
    return _PROGRAM_CACHE[cfg]


def kernel(user, item, rating_mtx, user_similarity, user_bias, item_bias, global_bias):
    from concourse import bass_utils

    cfg = Cfg()
    assert np.asarray(rating_mtx).shape == (cfg.n_users, cfg.n_items)
    assert np.asarray(user).shape == (cfg.batch,)
    nc_ubf, nc_main = _get_programs(cfg)
    g_lo, g_hi, A = make_tables(
        cfg, rating_mtx, user_similarity, user_bias, item_bias, global_bias
    )
    core_ids = list(range(cfg.n_cores))

    res1 = bass_utils.run_bass_kernel_spmd(
        nc_ubf, make_ubf_in_maps(cfg, A), core_ids=core_ids
    )
    # unshard the 8 user-slices of -(ubf-2.5) computed on device
    # (device layout [128, 8]: (p, j) = local user j*128+p)
    nubf = np.concatenate(
        [np.asarray(res1.results[k]["ubf"], np.float32)
         .reshape(128, cfg.ul // 128).T.ravel()
         for k in core_ids]
    )

    res2 = bass_utils.run_bass_kernel_spmd(
        nc_main,
        make_main_in_maps(cfg, user, item, g_lo, g_hi, A, nubf, user_bias),
        core_ids=core_ids,
    )
    out = np.concatenate(
        [np.asarray(res2.results[k]["out"], np.float32) for k in core_ids]
    )
    return out.reshape(cfg.batch)


# revision 40
# speedup vs baseline: 1.3072x; 1.0017x over previous
"""Trainium2 Bass kernel: collaborative-filtering score (segment_reduce).

Math (per batch element b):
    ubf[u]   = masked mean over nonzero entries of rating_mtx[u, :]
    score[b] = sum_u S[user_b, u] * (R[u, item_b] - ubf[u])
    out[b]   = 5 * sigmoid(score[b] + user_bias[user_b] + item_bias[item_b] + gb)

Distribution: BATCH-sharded, TWO launches.

Launch 1 (ubf): user axis sharded 8-way; each core scans its private
[4096 items x 1024 users] fp8 slice of (R.T - 2.5) and produces
-(ubf - 2.5) for its users via PE masked sum/count in fp32 PSUM. The 8
x [1024] slices are concatenated and re-laid-out on the HOST (pure
unshard/reshard of 32KB). A single-launch on-device exchange was built
and measured first: the collectives stack costs ~110us for the
auto-inserted kernel-entry barrier plus ~70us per op on this 8-core
mesh, so any collective-dependent compute cannot start before ~200us --
slower than this entire kernel.

Launch 2 (main): batch sharded 8-way. Core k owns batch slice
[k*1024,(k+1)*1024) and transpose-gathers FULL 8192-user rows per
(user_b, item_b) pair: G (bf16, three contiguous tables: the gather
ucode requires rows <=16KB and contiguous elem_step) and A (fp8 pairs
packed in int16, exact values +-{0.5,1.5,2.5}). Per 256-idx chunk, one
fused fp32-PSUM reduction accumulates [ubf-weighted G terms + hi/lo
bias columns + ones-weighted A(x)G products]; the product pass runs on
DVE (mixed fp8*bf16, single bf16 rounding) in place over gk after the
ubf pass has consumed it. sigmoid*5, DMA out the [1024] slice; the host
concatenates the 8 slices. Only 2048+few gather descriptors per core
(vs 16384 user-sharded), so GPSIMD descriptor generation is off the
critical path and the gathers stream at ~300GB/s.
"""

import sys
from dataclasses import dataclass

import numpy as np

if "/opt/trn_rl_repo" not in sys.path:
    sys.path.insert(0, "/opt/trn_rl_repo")


@dataclass(frozen=True)
class Cfg:
    n_users: int = 8192
    n_items: int = 4096
    batch: int = 8192
    n_cores: int = 8
    ch: int = 256  # gather idxs per chunk

    @property
    def bl(self) -> int:  # batch per core
        return self.batch // self.n_cores

    @property
    def ul(self) -> int:  # users per core (ubf launch)
        return self.n_users // self.n_cores

    @property
    def gfg(self) -> int:  # G data f-groups
        return self.n_users // 128

    @property
    def afu(self) -> int:  # A data unit f-groups (i16 units)
        return self.n_users // 256

    @property
    def awu(self) -> int:  # A row width (i16 units): data + 128-unit bias blk
        return self.n_users // 2 + 128


def build_ubf_program(cfg: Cfg):
    from concourse import bacc, mybir, tile

    f32 = mybir.dt.float32
    i16 = mybir.dt.int16
    fp8 = mybir.dt.float8e4
    Alu = mybir.AluOpType

    I, UL = cfg.n_items, cfg.ul
    USL = UL // 2  # slice width in i16 units
    NSUB = 4  # load sub-tiles (pipelining granularity)
    RT = I // 128  # item-rows per partition (32)
    RS = RT // NSUB  # item-rows per partition per sub-tile

    nc = bacc.Bacc(None, target_bir_lowering=False, debug=False)
    rsl_t = nc.dram_tensor("ratt_slice", [I, USL], i16, kind="ExternalInput")
    # [128, UL//128] layout: (p, j) holds -(ubf-2.5) for local user j*128+p
    out_t = nc.dram_tensor("ubf", [128, UL // 128], f32, kind="ExternalOutput")

    with tile.TileContext(nc) as tc:
        with (
            tc.tile_pool(name="static", bufs=1) as st,
            tc.tile_pool(name="rstream", bufs=2) as rpool,
            tc.tile_pool(name="maskp", bufs=2) as mpool,
            tc.tile_pool(name="ps", bufs=1, space="PSUM") as pp,
        ):
            # partition p of sub-tile s holds item rows (p*RT + s*RS ..
            # + RS): per-partition contiguous 8KB reads, full HBM bandwidth.
            rsl_v = rsl_t[:].rearrange("(p r) u -> p r u", p=128)
            # dual-fp8 ldweights wants the k-tile pair 16-element strided
            ones8 = st.tile([128, 2, 16], fp8)
            nc.gpsimd.memset(ones8[:], 1.0)
            ps_s = [pp.tile([1, 512], f32, name=f"ps_s{j}") for j in range(2)]
            ps_c = [pp.tile([1, 512], f32, name=f"ps_c{j}") for j in range(2)]
            DR = mybir.MatmulPerfMode.DoubleRow
            for s in range(NSUB):
                rt = rpool.tile([128, RS, USL], i16, name="rt")
                nc.sync.dma_start(
                    out=rt[:], in_=rsl_v[:, s * RS : (s + 1) * RS, :]
                )
                atf = rt[:].bitcast(fp8)  # [128, RS, 1024] users natural
                mk = mpool.tile([128, RS, UL], fp8, name="mk")
                nc.vector.tensor_scalar(
                    out=mk[:], in0=atf, scalar1=-2.5, scalar2=None,
                    op0=Alu.not_equal,
                )
                first, last = s == 0, s == NSUB - 1
                for r2 in range(RS // 2):
                    fr, lr = first and r2 == 0, last and r2 == RS // 2 - 1
                    for j in range(2):
                        nc.tensor.matmul(
                            out=ps_s[j][:], lhsT=ones8[:, :, 0:1],
                            rhs=atf[:, 2 * r2 : 2 * r2 + 2, j * 512 : (j + 1) * 512],
                            start=fr, stop=lr, perf_mode=DR,
                        )
                        nc.tensor.matmul(
                            out=ps_c[j][:], lhsT=ones8[:, :, 0:1],
                            rhs=mk[:, 2 * r2 : 2 * r2 + 2, j * 512 : (j + 1) * 512],
                            start=fr, stop=lr, perf_mode=DR,
                        )

            # scatter sums/counts to [128, 8] ((p,j) <- user j*128+p) via
            # K=1 matmuls so the elementwise finish runs 128 lanes wide
            # (a [1,1024] single-partition reciprocal alone costs ~7us).
            sum_row = st.tile([1, UL], f32)
            cnt_row = st.tile([1, UL], f32)
            for j in range(2):
                nc.vector.tensor_copy(
                    out=sum_row[:, j * 512 : (j + 1) * 512], in_=ps_s[j][:]
                )
                nc.vector.tensor_copy(
                    out=cnt_row[:, j * 512 : (j + 1) * 512], in_=ps_c[j][:]
                )
            one1 = st.tile([1, 1], f32)
            nc.gpsimd.memset(one1[:], 1.0)
            JC = UL // 128
            ps_sc = pp.tile([128, JC], f32, name="ps_sc", tag="ps_s0")
            ps_cc = pp.tile([128, JC], f32, name="ps_cc", tag="ps_c0")
            for j in range(JC):
                nc.tensor.matmul(
                    out=ps_sc[:, j : j + 1],
                    lhsT=sum_row[:, j * 128 : (j + 1) * 128],
                    rhs=one1[:], start=True, stop=True,
                )
                nc.tensor.matmul(
                    out=ps_cc[:, j : j + 1],
                    lhsT=cnt_row[:, j * 128 : (j + 1) * 128],
                    rhs=one1[:], start=True, stop=True,
                )
            # table is pre-centered: sum_A = sum(R) - 2.5*I, so
            # -(ubf - 2.5) = -((sum_A + 2.5*I)/max(cnt,1) - 2.5); cnt==0
            # rows give +2.5 = -(0 - 2.5), matching the reference's ubf=0.
            rcp = st.tile([128, JC], f32)
            nc.vector.tensor_scalar(
                out=rcp[:], in0=ps_cc[:], scalar1=1.0, scalar2=None,
                op0=Alu.max,
            )
            nc.vector.reciprocal(out=rcp[:], in_=rcp[:])
            ubf_loc = st.tile([128, JC], f32)
            nc.vector.scalar_tensor_tensor(
                out=ubf_loc[:], in0=ps_sc[:], scalar=2.5 * I, in1=rcp[:],
                op0=Alu.add, op1=Alu.mult,
            )
            nc.vector.tensor_scalar(
                out=ubf_loc[:], in0=ubf_loc[:], scalar1=2.5, scalar2=-1.0,
                op0=Alu.subtract, op1=Alu.mult,
            )
            nc.sync.dma_start(out=out_t[:], in_=ubf_loc[:])

    nc.compile()
    return nc


def build_main_program(cfg: Cfg):
    from concourse import bacc, mybir, tile

    f32 = mybir.dt.float32
    i16 = mybir.dt.int16
    bf16 = mybir.dt.bfloat16
    fp8 = mybir.dt.float8e4
    Alu = mybir.AluOpType
    Act = mybir.ActivationFunctionType

    U, I, BL = cfg.n_users, cfg.n_items, cfg.bl
    GFG, AFU, AWU = cfg.gfg, cfg.afu, cfg.awu
    # chunk schedule: small final chunks shrink the post-gather compute tail
    CHUNKS = [(0, 256), (256, 256), (512, 256), (768, 128), (896, 128)]
    NCH = len(CHUNKS)
    IDXC = BL // 16

    nc = bacc.Bacc(
        None, target_bir_lowering=False, debug=False, num_swdge_queues=4
    )

    glo_t = nc.dram_tensor("g_lo", [U, U // 2], bf16, kind="ExternalInput")
    ghi_t = nc.dram_tensor("g_hi", [U, U // 2], bf16, kind="ExternalInput")
    rtt_t = nc.dram_tensor("ratt_aug", [I, AWU], i16, kind="ExternalInput")
    ubb_t = nc.dram_tensor("ubb", [1, BL], f32, kind="ExternalInput")
    ubfc_t = nc.dram_tensor("ubf_ct", [128, GFG], bf16, kind="ExternalInput")
    uidx_t = nc.dram_tensor("uidx", [128, IDXC], i16, kind="ExternalInput")
    iidx_t = nc.dram_tensor("iidx", [128, IDXC], i16, kind="ExternalInput")
    out_t = nc.dram_tensor("out", [BL], f32, kind="ExternalOutput")

    with tile.TileContext(nc) as tc:
        with (
            tc.tile_pool(name="static", bufs=1) as st,
            tc.tile_pool(name="gpool", bufs=3) as gpool,
            tc.tile_pool(name="spool", bufs=2) as spool,
            tc.tile_pool(name="apool", bufs=2) as apool,
            tc.tile_pool(name="cpool", bufs=2) as cpool,
            tc.tile_pool(name="psum", bufs=2, space="PSUM") as pp,
        ):
            # ---- statics ----
            ones_b = st.tile([128, 1], bf16)
            nc.gpsimd.memset(ones_b[:], 1.0)
            uidx_sb = st.tile([128, IDXC], i16)
            nc.sync.dma_start(out=uidx_sb[:], in_=uidx_t[:])
            iidx_sb = st.tile([128, IDXC], i16)
            nc.sync.dma_start(out=iidx_sb[:], in_=iidx_t[:])
            ubf_ct = st.tile([128, GFG], bf16)
            nc.sync.dma_start(out=ubf_ct[:], in_=ubfc_t[:])
            ubb_sb = st.tile([1, BL], f32)
            nc.sync.dma_start(out=ubb_sb[:], in_=ubb_t[:])

            # ---- gathers: interleaved per chunk so A data lands with its G
            # counterpart (descriptor generation is serial on the gpsimd
            # engine; queue-0 ring-space waits pace it to the DMA rate, so
            # emitting all G first starves A until ~100us). apool holds all
            # chunks so A generation never blocks the queue. ----
            gks, aks = [], []
            H = GFG // 2
            for k, (off, ch) in enumerate(CHUNKS):
                if ch == 256:
                    gk = gpool.tile([128, GFG, ch], bf16, name="gk")
                    ak = apool.tile([128, AFU + 1, ch], i16, name="ak")
                else:
                    gk = spool.tile([128, GFG, ch], bf16, name="gks")
                    ak = spool.tile([128, AFU + 1, ch], i16, name="aks")
                idx = uidx_sb[:, off // 16 : (off + ch) // 16]
                iidx = iidx_sb[:, off // 16 : (off + ch) // 16]
                nc.gpsimd.dma_gather(
                    out_ap=gk[:, 0:H, :], in_ap=glo_t[:], idxs_ap=idx,
                    num_idxs=ch, num_idxs_reg=ch, elem_size=U // 2,
                    transpose=True, queue_num=0,
                )
                nc.gpsimd.dma_gather(
                    out_ap=ak[:], in_ap=rtt_t[:], idxs_ap=iidx,
                    num_idxs=ch, num_idxs_reg=ch, elem_size=AWU,
                    transpose=True, queue_num=1,
                )
                nc.gpsimd.dma_gather(
                    out_ap=gk[:, H : 2 * H, :], in_ap=ghi_t[:], idxs_ap=idx,
                    num_idxs=ch, num_idxs_reg=ch, elem_size=U // 2,
                    transpose=True, queue_num=2,
                )
                gks.append(gk)
                aks.append(ak)

            # ---- main loop ----
            # per chunk: ps2[1,CH] += sum_u -ubf_c[u]*G + bias columns (PE,
            # reads raw gk); then p = A(x)G overwrites gk in place (DVE);
            # then ps1[1,2*CH] += fg-pair-merged product sums (PE). Emission
            # is software-pipelined so P2(k+1) runs while DVE computes p(k).
            scores_row = st.tile([1, BL], f32)
            ps1_k, ps2_k = [None] * NCH, [None] * NCH

            def emit_pass2(k):
                # M=2 trick: lhsT [128,2] = [w_even | w_odd], rhs = the fg
                # pair flattened [128, 2*ch]; the diagonal blocks
                # out[0, 0:ch] and out[1, ch:2ch] hold the wanted sums (the
                # cross blocks accumulate harmlessly). Halves the matmul
                # count vs one mm per f-group.
                off, ch = CHUNKS[k]
                gk, ak = gks[k], aks[k]
                ps2 = pp.tile([2, 512], f32, name="ps2")
                ps2_k[k] = ps2
                for f2 in range(GFG // 2):
                    nc.tensor.matmul(
                        out=ps2[:, 0 : 2 * ch],
                        lhsT=ubf_ct[:, 2 * f2 : 2 * f2 + 2],
                        rhs=gk[:, 2 * f2 : 2 * f2 + 2, :].rearrange(
                            "p f i -> p (f i)"
                        ),
                        start=(f2 == 0), stop=False,
                    )
                nc.tensor.matmul(
                    out=ps2[0:1, 0:ch], lhsT=ones_b[:],
                    rhs=ak[:, AFU, :].bitcast(bf16),
                    start=False, stop=True,
                )

            def emit_prod(k):
                gk, ak = gks[k], aks[k]
                akf = ak[:, 0:AFU, :].bitcast(fp8).rearrange(
                    "p f (i b) -> p f b i", b=2
                )
                gkv = gk[:, 0:GFG, :].rearrange("p (f b) i -> p f b i", b=2)
                nc.vector.tensor_tensor(out=gkv, in0=akf, in1=gkv, op=Alu.mult)

            def emit_pass1(k):
                off, ch = CHUNKS[k]
                gk = gks[k]
                ps1 = pp.tile([1, 512], f32, name="ps1")
                ps1_k[k] = ps1
                for f2 in range(GFG // 2):
                    nc.tensor.matmul(
                        out=ps1[:, 0 : 2 * ch],
                        lhsT=ones_b[:],
                        rhs=gk[:, 2 * f2 : 2 * f2 + 2, :].rearrange(
                            "p f i -> p (f i)"
                        ),
                        start=(f2 == 0), stop=(f2 == GFG // 2 - 1),
                    )
                # score = ps1_even + ps1_odd + ps2[0,even] + ps2[1,odd].
                # Engines cannot start an access at partition 1, so bounce
                # ps2's row-1 block to partition 0 with a tiny SBUF DMA.
                sc = scores_row[:, off : off + ch]
                c1 = cpool.tile([1, 512], f32, name="c1")
                nc.vector.tensor_copy(out=c1[:, 0 : 2 * ch], in_=ps1[:, 0 : 2 * ch])
                c2 = cpool.tile([2, 512], f32, name="c2")
                nc.vector.tensor_copy(out=c2[:, 0 : 2 * ch], in_=ps2_k[k][:, 0 : 2 * ch])
                c2b = cpool.tile([1, 256], f32, name="c2b")
                nc.sync.dma_start(out=c2b[:, 0:ch], in_=c2[1:2, ch : 2 * ch])
                nc.vector.tensor_tensor(
                    out=sc, in0=c1[:, 0:ch], in1=c1[:, ch : 2 * ch], op=Alu.add
                )
                nc.vector.tensor_tensor(
                    out=sc, in0=sc, in1=c2[0:1, 0:ch], op=Alu.add
                )
                nc.vector.tensor_tensor(
                    out=sc, in0=sc, in1=c2b[:, 0:ch], op=Alu.add
                )

            emit_pass2(0)
            for k in range(1, NCH):
                emit_pass2(k)
                emit_prod(k - 1)
                emit_pass1(k - 1)
            emit_prod(NCH - 1)
            emit_pass1(NCH - 1)

            # ---- finish: + user_bias[user_b], sigmoid * 5 -> out slice ----
            nc.vector.tensor_tensor(
                out=scores_row[:], in0=scores_row[:], in1=ubb_sb[:], op=Alu.add
            )
            nc.scalar.activation(
                out=scores_row[:], in_=scores_row[:], func=Act.Sigmoid
            )
            nc.vector.tensor_scalar_mul(
                out=scores_row[:], in0=scores_row[:], scalar1=5.0
            )
            nc.sync.dma_start(
                out=out_t[:].rearrange("(o n) -> o n", o=1), in_=scores_row[:]
            )

    nc.compile()
    return nc


def make_ubf_in_maps(cfg, A):
    UL = cfg.ul
    return [
        {"ratt_slice": np.ascontiguousarray(
            A[:, k * (UL // 2) : (k + 1) * (UL // 2)])}
        for k in range(cfg.n_cores)
    ]


def make_tables(cfg, rating_mtx, user_similarity, user_bias, item_bias, global_bias):
    import ml_dtypes

    U, I = cfg.n_users, cfg.n_items
    sim = np.asarray(user_similarity, dtype=np.float32)
    R = np.asarray(rating_mtx, dtype=np.float32)
    ibg = np.asarray(item_bias, dtype=np.float32) + np.float32(np.asarray(global_bias))

    def hilo(x):
        hi = x.astype(ml_dtypes.bfloat16)
        lo = (x - hi.astype(np.float32)).astype(ml_dtypes.bfloat16)
        return hi, lo

    ib_hi, ib_lo = hilo(ibg)

    # G: pair-permuted columns (matches the fp8 16-bit transpose-gather
    # interleave), split into two contiguous halves (gather rows <16KB)
    c = np.arange(U)
    perm = 2 * ((c // 256) * 128 + (c % 128)) + ((c // 128) % 2)
    Gp = sim[:, perm].astype(ml_dtypes.bfloat16)
    g_lo = np.ascontiguousarray(Gp[:, : U // 2])
    g_hi = np.ascontiguousarray(Gp[:, U // 2 :])

    # A: (R.T - 2.5) packed fp8 pairs in i16 units + bf16 bias bit patterns
    A8 = np.ascontiguousarray((R.T - 2.5).astype(ml_dtypes.float8_e4m3fn))
    A = np.zeros((I, cfg.awu), np.int16)
    A[:, : U // 2] = A8.view(np.int16)
    A[:, U // 2] = ib_hi.view(np.int16)
    A[:, U // 2 + 1] = ib_lo.view(np.int16)
    return g_lo, g_hi, A


def make_main_in_maps(cfg, user, item, g_lo, g_hi, A, nubf, user_bias):
    import ml_dtypes

    U, BL, GFG = cfg.n_users, cfg.bl, cfg.gfg
    u_i = np.asarray(user).astype(np.int64)
    i_i = np.asarray(item).astype(np.int64)
    ub = np.asarray(user_bias, dtype=np.float32)
    # scatter -(ubf-2.5)[u] to the pair-permuted (p, fg=2*fu+b) layout
    ubf_ct = np.empty((128, GFG), np.float32)
    fu = np.arange(GFG) // 2
    b = np.arange(GFG) % 2
    p = np.arange(128)
    ubf_ct[:, :] = nubf[2 * (fu[None, :] * 128 + p[:, None]) + b[None, :]]
    ubf_ct = ubf_ct.astype(ml_dtypes.bfloat16)

    maps = []
    for k in range(cfg.n_cores):
        us = u_i[k * BL : (k + 1) * BL].astype(np.int16)
        it = i_i[k * BL : (k + 1) * BL].astype(np.int16)
        uidx = np.tile(us.reshape(BL // 16, 16).T, (8, 1))
        iidx = np.tile(it.reshape(BL // 16, 16).T, (8, 1))
        ubb = ub[u_i[k * BL : (k + 1) * BL]].reshape(1, BL)
        maps.append(
            {"g_lo": g_lo, "g_hi": g_hi, "ratt_aug": A, "ubb": ubb,
             "ubf_ct": ubf_ct, "uidx": uidx, "iidx": iidx}
        )
    return maps


_PROGRAM_CACHE = {}


def _get_programs(cfg: Cfg):
    if cfg not in _PROGRAM_CACHE:
        _PROGRAM_CACHE[cfg] = (build_ubf_program(cfg), build_main_program(cfg))
    return _PROGRAM_CACHE[cfg]


def kernel(user, item, rating_mtx, user_similarity, user_bias, item_bias, global_bias):
    from concourse import bass_utils

    cfg = Cfg()
    assert np.asarray(rating_mtx).shape == (cfg.n_users, cfg.n_items)
    assert np.asarray(user).shape == (cfg.batch,)
    nc_ubf, nc_main = _get_programs(cfg)
    g_lo, g_hi, A = make_tables(
        cfg, rating_mtx, user_similarity, user_bias, item_bias, global_bias
    )
    core_ids = list(range(cfg.n_cores))

    res1 = bass_utils.run_bass_kernel_spmd(
        nc_ubf, make_ubf_in_maps(cfg, A), core_ids=core_ids
    )
    # unshard the 8 user-slices of -(ubf-2.5) computed on device
    # (device layout [128, 8]: (p, j) = local user j*128+p)
    nubf = np.concatenate(
        [np.asarray(res1.results[k]["ubf"], np.float32)
         .reshape(128, cfg.ul // 128).T.ravel()
         for k in core_ids]
    )

    res2 = bass_utils.run_bass_kernel_spmd(
        nc_main,
        make_main_in_maps(cfg, user, item, g_lo, g_hi, A, nubf, user_bias),
        core_ids=core_ids,
    )
    out = np.concatenate(
        [np.asarray(res2.results[k]["out"], np.float32) for k in core_ids]
    )
    return out.reshape(cfg.batch)
